# revision 1
# baseline (speedup 1.0000x reference)
import sys

sys.path.insert(0, "/opt/trn_rl_repo")

import numpy as np

N = 50000
E = 800000
F_IN = 64
H = 128
L = 3
C = 2
NEG = 0.2
EPS = 1e-5
NCORES = 8
NSHARD = N // NCORES  # 6250


def _bass_input_proj(x, Win, bin_):
    """relu(x @ Win + bin_) on 8 NeuronCores, node-sharded."""
    from concourse import bass_utils, tile
    from concourse.bass import Bass, MemorySpace
    import concourse.bass as bass
    from concourse import mybir

    f32 = mybir.dt.float32
    nc = Bass()

    xt_d = nc.dram_tensor("xt", (F_IN, NSHARD), f32, kind="ExternalInput")
    w_d = nc.dram_tensor("w", (F_IN, H), f32, kind="ExternalInput")
    b_d = nc.dram_tensor("b", (H, 1), f32, kind="ExternalInput")
    out_d = nc.dram_tensor("out", (H, NSHARD), f32, kind="ExternalOutput")

    FT = 512
    with tile.TileContext(nc) as tc:
        with (
            tc.tile_pool(name="pool", bufs=2) as pool,
            tc.tile_pool(name="psum", bufs=2, space=MemorySpace.PSUM) as psum,
        ):
            xt_s = pool.tile((F_IN, NSHARD), f32)
            w_s = pool.tile((F_IN, H), f32)
            b_s = pool.tile((H, 1), f32)
            out_s = pool.tile((H, NSHARD), f32)
            nc.gpsimd.dma_start(xt_s[:], xt_d[:])
            nc.gpsimd.dma_start(w_s[:], w_d[:])
            nc.gpsimd.dma_start(b_s[:], b_d[:])
            for f0 in range(0, NSHARD, FT):
                fw = min(FT, NSHARD - f0)
                pt = psum.tile((H, FT), f32)
                nc.tensor.matmul(pt[:, :fw], xt_s[:, f0 : f0 + fw], w_s[:])
                nc.scalar.activation(
                    out_s[:, f0 : f0 + fw],
                    pt[:, :fw],
                    mybir.ActivationFunctionType.Relu,
                    bias=b_s[:],
                )
            nc.gpsimd.dma_start(out_d[:], out_s[:])

    in_maps = []
    for c in range(NCORES):
        sh = np.ascontiguousarray(x[c * NSHARD : (c + 1) * NSHARD].T)
        in_maps.append(
            {
                "xt": sh,
                "w": np.ascontiguousarray(Win),
                "b": np.ascontiguousarray(bin_.reshape(H, 1)),
            }
        )
    res = bass_utils.run_bass_kernel_spmd(nc, in_maps, list(range(NCORES))).results
    h = np.concatenate([np.asarray(res[c]["out"]).T for c in range(NCORES)], axis=0)
    return h.astype(np.float32)


def kernel(x, edge_index, Win, bin_, Wl, bl, Wr, br, att, bg, ln_g, ln_b, W1, b1, W2, b2):
    x = np.asarray(x, np.float32)
    try:
        h0 = _bass_input_proj(x, np.asarray(Win, np.float32), np.asarray(bin_, np.float32))
    except Exception as e:
        print(f"[kernel] bass path failed ({e!r}); host fallback", file=sys.stderr)
        h0 = np.maximum(x @ np.asarray(Win, np.float32) + np.asarray(bin_, np.float32), 0.0)

    import jax
    import jax.numpy as jnp

    cpu = jax.devices("cpu")[0]
    with jax.default_device(cpu):
        loops = jnp.arange(N, dtype=jnp.int32)
        src = jnp.concatenate([jnp.asarray(edge_index[0], jnp.int32), loops])
        dst = jnp.concatenate([jnp.asarray(edge_index[1], jnp.int32), loops])
        h = jnp.asarray(h0)
        res = h
        for i in range(L):
            xl = h @ jnp.asarray(Wl[i]) + jnp.asarray(bl[i])
            xr = h @ jnp.asarray(Wr[i]) + jnp.asarray(br[i])
            e = jax.nn.leaky_relu(xl[src] + xr[dst], NEG)
            s = e @ jnp.asarray(att[i])
            m = jax.ops.segment_max(s, dst, num_segments=N)
            w = jnp.exp(s - m[dst])
            z = jax.ops.segment_sum(w, dst, num_segments=N)
            alpha = w / z[dst]
            out = jax.ops.segment_sum(xl[src] * alpha[:, None], dst, num_segments=N) + jnp.asarray(bg[i])
            if i > 0:
                out = out + res
            mu = out.mean(-1, keepdims=True)
            var = ((out - mu) ** 2).mean(-1, keepdims=True)
            out = (out - mu) * jax.lax.rsqrt(var + EPS) * jnp.asarray(ln_g[i]) + jnp.asarray(ln_b[i])
            if i < L - 1:
                out = jax.nn.relu(out)
            h = out
            res = h
        y = jax.nn.relu(h @ jnp.asarray(W1) + jnp.asarray(b1)) @ jnp.asarray(W2) + jnp.asarray(b2)
        return np.asarray(y, np.float32)



# revision 21
# speedup vs baseline: 1.3960x; 1.3960x over previous
import sys

sys.path.insert(0, "/opt/trn_rl_repo")

import numpy as np

N = 50000
E = 800000
F_IN = 64
H = 128
L = 3
C = 2
NEG = 0.2
EPS = 1e-5
NCORES = 8
NSHARD = N // NCORES            # 6250
TILES = (NSHARD + 127) // 128   # 49
NPAD = TILES * 128              # 6272 padded shard size
NFULL = NCORES * NPAD           # 50176 padded gather-table rows
LO_ROWS = 32768                 # int16 index limit
HI_BASE = NFULL - 32768         # 17408

TABLE_BF16 = False              # gather tables / weights / h_T in bf16
LAST_EXEC_NS = None
LAST_RESULTS = None
import os as _os
DEV_MODE = int(_os.environ.get("GAT_DEV_MODE", "4"))
DEV_SKIP = set(_os.environ.get("GAT_SKIP", "").split(","))


def _prep_graph(edge_index):
    """Bucket edges by (core, dst tile), split by src table row (lo/hi),
    pad each group to 128-multiples, build gather-idx / dstloc blobs."""
    src = edge_index[0].astype(np.int64)
    dst = edge_index[1].astype(np.int64)
    # self loops for every real node
    loops = np.arange(N, dtype=np.int64)
    src = np.concatenate([src, loops])
    dst = np.concatenate([dst, loops])
    # fake edges (src=0) for pad nodes of each core so z > 0
    pad_dst = []
    for c in range(NCORES):
        for l in range(NSHARD, NPAD):
            pad_dst.append(c * NSHARD + l)  # virtual: maps to (c, local l)
    # handle pad separately (their "dst" ids exceed real mapping)
    # table row of src node (node g owned by core g//NSHARD at local g%NSHARD)
    srow = (src // NSHARD) * NPAD + (src % NSHARD)
    core = dst // NSHARD
    local = dst % NSHARD
    tile = local // 128
    dloc = local % 128

    # pad-node fake edges
    p_core = np.repeat(np.arange(NCORES), NPAD - NSHARD)
    p_tile = np.full(p_core.shape, TILES - 1, np.int64)
    p_dloc = np.tile(np.arange(NSHARD - (TILES - 1) * 128, NPAD - (TILES - 1) * 128), NCORES)
    p_srow = np.zeros(p_core.shape, np.int64)

    core = np.concatenate([core, p_core])
    tile = np.concatenate([tile, p_tile])
    dloc = np.concatenate([dloc, p_dloc])
    srow = np.concatenate([srow, p_srow])

    hi = (srow >= LO_ROWS).astype(np.int64)
    # sort edges by (core, tile, hi)
    key = ((core * TILES + tile) * 2 + hi)
    order = np.argsort(key, kind="stable")
    srow = srow[order]
    dloc = dloc[order]
    key = key[order]

    # counts per (core, tile, hi)
    cnt = np.bincount(key, minlength=NCORES * TILES * 2).reshape(NCORES, TILES, 2)
    ch_lo = (cnt[:, :, 0] + 127) // 128    # chunks needed per (core, tile)
    ch_hi = (cnt[:, :, 1] + 127) // 128
    C_LO = ch_lo.max(axis=0)               # per-tile, max over cores
    C_HI = ch_hi.max(axis=0)
    C_T = C_LO + C_HI                      # chunks per tile

    # slot arrays per core
    tot_chunks = int(C_T.sum())
    slot_srow = np.zeros((NCORES, tot_chunks * 128), np.int64)
    slot_dloc = np.full((NCORES, tot_chunks * 128), -1.0, np.float32)

    # chunk offset of each tile
    chunk_off = np.zeros(TILES + 1, np.int64)
    chunk_off[1:] = np.cumsum(C_T)

    # cumulative start of each (core,tile,hi) group in sorted arrays
    grp_start = np.zeros(NCORES * TILES * 2 + 1, np.int64)
    grp_start[1:] = np.cumsum(cnt.reshape(-1))

    for c in range(NCORES):
        for t in range(TILES):
            base = chunk_off[t] * 128
            for h_ in (0, 1):
                g = (c * TILES + t) * 2 + h_
                s0, s1 = grp_start[g], grp_start[g + 1]
                n = s1 - s0
                off = base if h_ == 0 else base + int(C_LO[t]) * 128
                slot_srow[c, off:off + n] = srow[s0:s1]
                slot_dloc[c, off:off + n] = dloc[s0:s1]

    # build idx blobs in dma_gather layout: [128, cols]; idx j at [j%16, j//16],
    # replicated across the 8 Q7 core stripes
    def pack_idx(vals):  # vals [n] (n % 128 == 0) -> [128, n//16] int16
        n = vals.shape[0]
        a = vals.reshape(n // 16, 16).T.astype(np.int16)  # [16, n//16]
        return np.tile(a, (8, 1))

    # per tile: xl-lo idx | xl-hi idx | xr idx , concatenated across tiles
    idx_blob = []
    dl_blob = []
    for c in range(NCORES):
        cols = []
        for t in range(TILES):
            s0 = chunk_off[t] * 128
            nlo = int(C_LO[t]) * 128
            nhi = int(C_HI[t]) * 128
            rows = slot_srow[c, s0:s0 + nlo + nhi]
            dl = slot_dloc[c, s0:s0 + nlo + nhi]
            lo_idx = rows[:nlo]
            hi_idx = rows[nlo:] - HI_BASE
            hi_idx = np.where(dl[nlo:] < 0, 0, hi_idx)  # pads -> 0
            xr_idx = (t * 128 + np.where(dl < 0, 0, dl)).astype(np.int64)
            cols.append(pack_idx(lo_idx))
            cols.append(pack_idx(hi_idx))
            cols.append(pack_idx(xr_idx))
        idx_blob.append(np.concatenate(cols, axis=1))
        # dstloc blob: [128, tot_chunks] f32, entry [p, chunk] = dloc of slot
        dl_blob.append(np.ascontiguousarray(
            slot_dloc[c].reshape(tot_chunks, 128).T))
    idx_blob = np.stack(idx_blob)          # [NCORES, 128, idxcols]
    dl_blob = np.stack(dl_blob)            # [NCORES, 128, tot_chunks]

    return dict(C_LO=C_LO.astype(int), C_HI=C_HI.astype(int), C_T=C_T.astype(int),
                chunk_off=chunk_off, idx_blob=idx_blob, dl_blob=dl_blob)


def _build_program(meta, have_bias):
    from concourse import tile, bacc
    from concourse import mybir

    f32 = mybir.dt.float32
    bf16 = mybir.dt.bfloat16
    tdt = bf16 if TABLE_BF16 else f32

    C_LO, C_HI, C_T = meta["C_LO"], meta["C_HI"], meta["C_T"]
    TOT_CHUNKS = int(C_T.sum())
    IDXCOLS = int(meta["idx_blob"].shape[2])
    CMAX = int(C_T.max())

    nc = bacc.Bacc("TRN2", num_devices=NCORES, num_swdge_queues=4)

    # ---------------- dram tensors ----------------
    xT_d = nc.dram_tensor("xT", (F_IN, NPAD), tdt, kind="ExternalInput")
    idx_d = nc.dram_tensor("idx", (128, IDXCOLS), mybir.dt.int16, kind="ExternalInput")
    dl_d = nc.dram_tensor("dl", (128, TOT_CHUNKS), f32, kind="ExternalInput")
    # weights blob (table dtype): Win|Wl0..2|Wr0..2|W1|W2 as columns
    WCOLS = H * (1 + L + L) + 64 + 2
    wb_d = nc.dram_tensor("wb", (128, WCOLS), tdt, kind="ExternalInput")
    # f32 consts blob: identity | iota_rep | att_rep(3) | per-partition cols:
    #   bin_(128)|b1(64)|b2(2) cols -> pack as [128, 3] tail
    CCOLS = 128 * 2 + 4
    cb_d = nc.dram_tensor("cb", (128, CCOLS), f32, kind="ExternalInput")
    att_d = nc.dram_tensor("attb", (128, max(1, 128 * L)), tdt, kind="ExternalInput")
    y_d = nc.dram_tensor("y", (C, NPAD), f32, kind="ExternalOutput")
    hdbg_d = (nc.dram_tensor("hdbg", (L, 128, TILES * H), f32, kind="ExternalOutput")
              if DEV_MODE != 4 or _os.environ.get("GAT_DBG") else None)
    xldbg_d = (nc.dram_tensor("xldbg", (L, NPAD, H), tdt, kind="ExternalOutput")
               if _os.environ.get("GAT_DBG") else None)
    gdbg_d = (nc.dram_tensor("gdbg", (2, 128, CMAX * H), tdt, kind="ExternalOutput")
              if _os.environ.get("GAT_DBG") else None)
    wdbg_d = (nc.dram_tensor("wdbg", (128, CMAX), tdt, kind="ExternalOutput")
              if _os.environ.get("GAT_DBG") else None)
    mdbg_d = (nc.dram_tensor("mdbg", (3, 128, H), f32, kind="ExternalOutput")
              if _os.environ.get("GAT_DBG") else None)

    xl_sh_d = [nc.dram_tensor(f"xl_sh{i}", (NPAD, H), tdt, kind="Internal")
               for i in range(2)]
    xr_sh_d = [nc.dram_tensor(f"xr_sh{i}", (NPAD, H), tdt, kind="Internal")
               for i in range(2)]
    _xl_full_single = nc.dram_tensor("xl_full0", (NFULL, H), tdt, kind="Internal",
                                     addr_space="Shared")
    xl_full_d = [_xl_full_single, _xl_full_single]

    ID_O = 0
    IOTA_O = 128
    BIAS_O = 256

    with tile.TileContext(nc) as tc:
        with (
            tc.tile_pool(name="const", bufs=1) as constp,
            tc.tile_pool(name="persist", bufs=1) as persist,
            tc.tile_pool(name="work", bufs=2) as work,
            tc.tile_pool(name="gat", bufs=3) as gatp,
            tc.tile_pool(name="psA", bufs=2, space="PSUM") as psA,
            tc.tile_pool(name="psB", bufs=2, space="PSUM") as psB,
            tc.tile_pool(name="psC", bufs=2, space="PSUM") as psC,
        ):
            # ---------- load constants ----------
            wb = constp.tile((128, WCOLS), tdt)
            cb = constp.tile((128, CCOLS), f32)
            attb = constp.tile((128, max(1, 128 * L)), tdt)
            nc.sync.dma_start(wb[:], wb_d[:])
            nc.sync.dma_start(cb[:], cb_d[:])
            nc.sync.dma_start(attb[:], att_d[:])
            ident = cb[:, ID_O:ID_O + 128]
            iota = cb[:, IOTA_O:IOTA_O + 128]
            Win_w = wb[:F_IN, 0:H]
            Wl_w = [wb[:, H * (1 + i):H * (2 + i)] for i in range(L)]
            Wr_w = [wb[:, H * (1 + L + i):H * (2 + L + i)] for i in range(L)]
            W1_w = wb[:, H * (1 + 2 * L):H * (1 + 2 * L) + 64]
            W2_w = wb[:64, H * (1 + 2 * L) + 64:H * (1 + 2 * L) + 66]
            bin_c = cb[:, BIAS_O:BIAS_O + 1]
            b1_c = cb[:64, BIAS_O + 1:BIAS_O + 2]
            b2_c = cb[:C, BIAS_O + 2:BIAS_O + 3]
            eps_c = cb[:, BIAS_O + 3:BIAS_O + 4]

            ones_col = nc.const_aps.tensor(1.0, (128, 1), tdt)

            # primer matmuls: absorb const DMA-queue sems into PE clock
            prim = psC.tile((1, 1), f32, tag="prim")
            nc.tensor.matmul(prim[:], wb[:, 0:1], wb[:, 0:1], start=True, stop=True)
            prim2 = psC.tile((1, 1), f32, tag="prim")
            nc.tensor.matmul(prim2[:], cb[:, 0:1], cb[:, 0:1], start=True, stop=True)
            prim3 = psC.tile((1, 1), f32, tag="prim")
            nc.tensor.matmul(prim3[:], ones_col, ones_col, start=True, stop=True)

            # ---------- input projection: h0_T = relu(Win.T @ xT + bin) ----------
            xTc = persist.tile((F_IN, NPAD), tdt)
            nc.sync.dma_start(xTc[:], xT_d[:])
            hT = persist.tile((H, NPAD), tdt)
            vA = persist.tile((128, TILES, H), f32)
            vB = persist.tile((128, TILES, H), f32)
            FT = 512
            for o in range(0, NPAD, FT):
                w_ = min(FT, NPAD - o)
                pj = psA.tile((H, FT), f32, tag="wide")
                nc.tensor.matmul(pj[:, :w_], Win_w, xTc[:, o:o + w_], start=True, stop=True)
                nc.scalar.activation(hT[:, o:o + w_], pj[:, :w_],
                                     mybir.ActivationFunctionType.Relu, bias=bin_c)

            # persistent dstloc for message phase
            dlb = persist.tile((128, TOT_CHUNKS), f32)
            nc.sync.dma_start(dlb[:], dl_d[:])

            vbufs = [vA, vB]
            for li in range(L):
                vout = vbufs[li % 2]
                res = vbufs[(li + 1) % 2]
                xl_sh = xl_sh_d[li % 2]
                xr_sh = xr_sh_d[li % 2]
                xl_full = xl_full_d[li % 2]
                att_rep = attb[:, 128 * li:128 * (li + 1)]

                # ---- xl/xr shard tables (node-major) + push to HBM ----
                for t in range(TILES if "xw" not in DEV_SKIP else 0):
                    hTt = hT[:, t * 128:(t + 1) * 128]
                    pxl = psB.tile((128, H), f32, tag="nm")
                    nc.tensor.matmul(pxl[:], hTt, Wl_w[li], start=True, stop=True)
                    xl_sb = work.tile((128, H), tdt, tag="xl")
                    nc.scalar.copy(xl_sb[:], pxl[:])
                    nc.sync.dma_start(
                        xl_sh[t * 128:(t + 1) * 128, :], xl_sb[:])
                    if xldbg_d is not None:
                        nc.sync.dma_start(
                            xldbg_d[li, t * 128:(t + 1) * 128, :], xl_sb[:])
                    pxr = psB.tile((128, H), f32, tag="nm")
                    nc.tensor.matmul(pxr[:], hTt, Wr_w[li], start=True, stop=True)
                    xr_sb = work.tile((128, H), tdt, tag="xr")
                    nc.scalar.copy(xr_sb[:], pxr[:])
                    nc.sync.dma_start(
                        xr_sh[t * 128:(t + 1) * 128, :], xr_sb[:])

                # ---- allgather xl table ----
                if "ag" in DEV_SKIP:
                    pass
                else:
                    nc.gpsimd.collective_compute(
                    "AllGather",
                    mybir.AluOpType.bypass,
                        replica_groups=[list(range(NCORES))],
                        ins=[xl_sh[:].opt()],
                        outs=[xl_full[:].opt()],
                    )

                # ---- message passing per dst tile ----
                icol = 0
                for t in range(TILES):
                    if DEV_MODE == 1:
                        pd = psB.tile((128, H), f32, tag="nm")
                        nc.tensor.matmul(pd[:], hT[:, t * 128:(t + 1) * 128],
                                         Wl_w[li], start=True, stop=True)
                        nc.vector.tensor_scalar(vout[:, t, :], pd[:], 1.0, None,
                                                mybir.AluOpType.mult)
                        continue
                    clo, chi, ct = int(C_LO[t]), int(C_HI[t]), int(C_T[t])
                    co = int(meta["chunk_off"][t])
                    nlo, nhi, nt = clo * 128, chi * 128, ct * 128
                    G = gatp.tile((128, CMAX, H), tdt, tag="G")
                    Gx = gatp.tile((128, CMAX, H), tdt, tag="Gx")
                    itile = gatp.tile((128, 2 * CMAX * 8), mybir.dt.int16, tag="idx")
                    ic = 2 * nt // 16
                    nc.sync.dma_start(itile[:, :ic], idx_d[:, icol:icol + ic])
                    # dma_gather hangs HW for num_idxs > 1024: split into
                    # <=8-chunk pieces, round-robin over 4 SWDGE queues
                    qn = [0]

                    def gat(dst_ap, src_ap, col0, chunks, chunk0):
                        for p0 in range(0, chunks, 8):
                            pw = min(8, chunks - p0)
                            nc.gpsimd.dma_gather(
                                dst_ap[:, chunk0 + p0:chunk0 + p0 + pw, :],
                                src_ap,
                                itile[:, col0 + p0 * 8:col0 + (p0 + pw) * 8],
                                pw * 128, pw * 128, H, queue_num=qn[0] % 4)
                            qn[0] += 1

                    if clo:
                        gat(G, xl_full[:LO_ROWS, :], 0, clo, 0)
                    if chi:
                        gat(G, xl_full[HI_BASE:, :], nlo // 16, chi, clo)
                    gat(Gx, xr_sh[:], (nlo + nhi) // 16, ct, 0)
                    icol += ic

                    if DEV_MODE == 2:
                        if "fil" not in DEV_SKIP:
                            pd = psB.tile((128, H), f32, tag="nm")
                            nc.tensor.matmul(pd[:], hT[:, t * 128:(t + 1) * 128],
                                             Wl_w[li], start=True, stop=True)
                            nc.vector.tensor_scalar(vout[:, t, :], pd[:], 1.0, None,
                                                    mybir.AluOpType.mult)
                        nc.vector.tensor_tensor(Gx[:, 0, :], Gx[:, 0, :],
                                                G[:, 0, :], mybir.AluOpType.add)
                        continue
                    if gdbg_d is not None and li == 1 and t == 0:
                        nc.sync.dma_start(gdbg_d[0], G[:].rearrange("p c h -> p (c h)"))
                        nc.sync.dma_start(gdbg_d[1], Gx[:].rearrange("p c h -> p (c h)"))
                    dl_t = dlb[:, co:co + ct]
                    oh = gatp.tile((128, CMAX, 128), tdt, tag="oh")
                    nc.vector.tensor_tensor(
                        oh[:, :ct, :],
                        iota.unsqueeze(1).broadcast_to((128, ct, 128)),
                        dl_t.unsqueeze(2).broadcast_to((128, ct, 128)),
                        mybir.AluOpType.is_equal)
                    # T0 = G + Gx (in place in Gx)
                    nc.vector.tensor_tensor(Gx[:, :ct, :], G[:, :ct, :],
                                            Gx[:, :ct, :], mybir.AluOpType.add)
                    # lrelu in place: max(0.2*x, x)
                    nc.vector.scalar_tensor_tensor(
                        Gx[:, :ct, :], Gx[:, :ct, :], NEG, Gx[:, :ct, :],
                        mybir.AluOpType.mult, mybir.AluOpType.max)
                    # s = sum(lrelu * att) per chunk via fused accum
                    sco = work.tile((128, CMAX), f32, tag="s")
                    for cc in range(ct):
                        nc.vector.scalar_tensor_tensor(
                            Gx[:, cc, :], Gx[:, cc, :], 1.0, att_rep,
                            mybir.AluOpType.mult, mybir.AluOpType.mult,
                            accum_out=sco[:, cc:cc + 1])
                    wexp = work.tile((128, CMAX), tdt, tag="w")
                    nc.scalar.activation(wexp[:, :ct], sco[:, :ct],
                                         mybir.ActivationFunctionType.Exp)
                    # wG in place in G
                    nc.vector.tensor_tensor(
                        G[:, :ct, :], G[:, :ct, :],
                        wexp[:, :ct].unsqueeze(2).broadcast_to((128, ct, 128)),
                        mybir.AluOpType.mult)
                    if DEV_MODE == 3:
                        pd = psB.tile((128, H), f32, tag="nm")
                        nc.tensor.matmul(pd[:], hT[:, t * 128:(t + 1) * 128],
                                         Wl_w[li], start=True, stop=True)
                        nc.vector.tensor_scalar(vout[:, t, :], pd[:], 1.0, None,
                                                mybir.AluOpType.mult)
                        continue
                    pnf = psA.tile((128, H), f32, tag="wide")
                    pz = psB.tile((128, 1), f32, tag="nm")
                    for cc in range(ct):
                        nc.tensor.matmul(pnf[:], oh[:, cc, :], G[:, cc, :],
                                         start=(cc == 0), stop=(cc == ct - 1))
                    for cc in range(ct):
                        nc.tensor.matmul(pz[:], oh[:, cc, :], wexp[:, cc:cc + 1],
                                         start=(cc == 0), stop=(cc == ct - 1))
                    rz = work.tile((128, 1), f32, tag="rz")
                    nc.vector.reciprocal(rz[:], pz[:])
                    if mdbg_d is not None and li == 1 and t == 0:
                        nc.sync.dma_start(wdbg_d[:], wexp[:])
                        mnum = work.tile((128, H), f32, tag="mnum")
                        nc.vector.tensor_copy(mnum[:], pnf[:])
                        nc.sync.dma_start(mdbg_d[0], mnum[:])
                        mz = work.tile((128, H), f32, tag="mnum")
                        nc.vector.memset(mz[:], 0.0)
                        nc.vector.tensor_copy(mz[:, 0:1], pz[:])
                        nc.vector.tensor_copy(mz[:, 1:2], rz[:])
                        nc.sync.dma_start(mdbg_d[1], mz[:])
                    nc.vector.tensor_scalar(vout[:, t, :], pnf[:], rz[:], None,
                                            mybir.AluOpType.mult)

                # ---- residual + layernorm (+relu) batched ----
                if "ln" in DEV_SKIP:
                    for t in range(TILES):
                        ptr = psC.tile((128, 128), f32, tag="tr")
                        nc.tensor.matmul(ptr[:], vout[:, t, :], ident,
                                         start=True, stop=True)
                        nc.scalar.copy(hT[:, t * 128:(t + 1) * 128], ptr[:])
                    continue
                vflat = vout[:].rearrange("p t h -> p (t h)")
                if li > 0:
                    nc.vector.tensor_tensor(vflat, vflat,
                                            res[:].rearrange("p t h -> p (t h)"),
                                            mybir.AluOpType.add)
                mean = work.tile((128, TILES), f32, tag="mean")
                nc.vector.tensor_reduce(mean[:], vout[:], mybir.AxisListType.X,
                                        mybir.AluOpType.add)
                mean2 = work.tile((128, TILES), f32, tag="mean2")
                nc.scalar.activation(mean2[:], mean[:],
                                     mybir.ActivationFunctionType.Copy,
                                     scale=1.0 / H)
                nc.vector.tensor_tensor(
                    vout[:], vout[:],
                    mean2[:].unsqueeze(2).broadcast_to((128, TILES, H)),
                    mybir.AluOpType.subtract)
                var = work.tile((128, TILES), f32, tag="var")
                sqjunk = work.tile((128, H), f32, tag="sqj")
                for t in range(TILES):
                    nc.scalar.activation(sqjunk[:], vout[:, t, :],
                                         mybir.ActivationFunctionType.Square,
                                         accum_out=var[:, t:t + 1])
                std = work.tile((128, TILES), f32, tag="std")
                nc.scalar.activation(std[:], var[:],
                                     mybir.ActivationFunctionType.Sqrt,
                                     bias=eps_c, scale=1.0 / H)
                rstd = work.tile((128, TILES), f32, tag="rstd")
                nc.vector.reciprocal(rstd[:], std[:])
                nc.vector.tensor_tensor(
                    vout[:], vout[:],
                    rstd[:].unsqueeze(2).broadcast_to((128, TILES, H)),
                    mybir.AluOpType.mult)
                if li < L - 1:
                    nc.scalar.activation(vflat, vflat,
                                         mybir.ActivationFunctionType.Relu)

                if hdbg_d is not None:
                    nc.sync.dma_start(hdbg_d[li], vout[:].rearrange("p t h -> p (t h)"))
                # ---- transpose v -> hT for next stage ----
                for t in range(TILES):
                    ptr = psC.tile((128, 128), f32, tag="tr")
                    nc.tensor.matmul(ptr[:], vout[:, t, :], ident,
                                     start=True, stop=True)
                    nc.scalar.copy(hT[:, t * 128:(t + 1) * 128], ptr[:])

            # ---------- output MLP ----------
            for o in range(0, NPAD, FT):
                w_ = min(FT, NPAD - o)
                p1 = psA.tile((64, FT), f32, tag="wide")
                nc.tensor.matmul(p1[:, :w_], W1_w, hT[:, o:o + w_], start=True, stop=True)
                y1 = work.tile((64, FT), tdt, tag="y1")
                nc.scalar.activation(y1[:, :w_], p1[:, :w_],
                                     mybir.ActivationFunctionType.Relu, bias=b1_c)
                p2 = psB.tile((C, FT), f32, tag="nm")
                nc.tensor.matmul(p2[:, :w_], W2_w, y1[:64, :w_], start=True, stop=True)
                y2 = work.tile((C, FT), f32, tag="y2")
                nc.scalar.activation(y2[:, :w_], p2[:, :w_],
                                     mybir.ActivationFunctionType.Identity, bias=b2_c)
                nc.sync.dma_start(y_d[:, o:o + w_], y2[:, :w_])

    nc.finalize()
    return nc


def _run_device(inputs, meta):
    from concourse import bass_utils

    x = np.asarray(inputs["x"], np.float32)
    Win = np.asarray(inputs["Win"], np.float32)
    bin_ = np.asarray(inputs["bin_"], np.float32)
    Wl = np.asarray(inputs["Wl"], np.float32)
    Wr = np.asarray(inputs["Wr"], np.float32)
    att = np.asarray(inputs["att"], np.float32)
    W1 = np.asarray(inputs["W1"], np.float32)
    b1 = np.asarray(inputs["b1"], np.float32)
    W2 = np.asarray(inputs["W2"], np.float32)
    b2 = np.asarray(inputs["b2"], np.float32)

    tdt_np = np.dtype("float32")
    if TABLE_BF16:
        import ml_dtypes
        tdt_np = np.dtype(ml_dtypes.bfloat16)

    # weights blob
    WCOLS = H * (1 + L + L) + 64 + 2
    wb = np.zeros((128, WCOLS), np.float32)
    wb[:F_IN, 0:H] = Win
    for i in range(L):
        wb[:, H * (1 + i):H * (2 + i)] = Wl[i]
        wb[:, H * (1 + L + i):H * (2 + L + i)] = Wr[i]
    wb[:, H * (1 + 2 * L):H * (1 + 2 * L) + 64] = W1
    wb[:64, H * (1 + 2 * L) + 64:H * (1 + 2 * L) + 66] = W2
    wb = wb.astype(tdt_np)

    # consts blob
    CCOLS = 128 * 2 + 4
    cb = np.zeros((128, CCOLS), np.float32)
    cb[:, 0:128] = np.eye(128, dtype=np.float32)
    cb[:, 128:256] = np.broadcast_to(np.arange(128, dtype=np.float32), (128, 128))
    BIAS_O = 256
    cb[:, BIAS_O] = bin_
    cb[:64, BIAS_O + 1] = b1
    cb[:C, BIAS_O + 2] = b2
    cb[:, BIAS_O + 3] = EPS
    attb = (np.concatenate(
        [np.broadcast_to(att[i], (128, 128)) for i in range(L)], axis=1).astype(tdt_np)
        if L else np.zeros((128, 1), tdt_np))

    nc = _build_program(meta, have_bias=False)

    in_maps = []
    for c in range(NCORES):
        xs = np.zeros((F_IN, NPAD), tdt_np)
        xs[:, :NSHARD] = x[c * NSHARD:(c + 1) * NSHARD].T.astype(tdt_np)
        in_maps.append({
            "xT": np.ascontiguousarray(xs),
            "idx": np.ascontiguousarray(meta["idx_blob"][c]),
            "dl": np.ascontiguousarray(meta["dl_blob"][c]),
            "wb": np.ascontiguousarray(wb),
            "cb": np.ascontiguousarray(cb),
            "attb": np.ascontiguousarray(attb),
        })
    trace = bool(_os.environ.get("GAT_TRACE"))
    res = bass_utils.run_bass_kernel_spmd(
        nc, in_maps, list(range(NCORES)), trace=trace,
        trace_cores=list(range(NCORES)) if trace else None)
    global LAST_EXEC_NS, LAST_RESULTS
    LAST_EXEC_NS = res.exec_time_ns
    LAST_RESULTS = res
    y = np.concatenate(
        [np.asarray(res.results[c]["y"]).T[:NSHARD] for c in range(NCORES)], axis=0)
    return np.ascontiguousarray(y.astype(np.float32))


def _host_fallback(inputs):
    import jax
    import jax.numpy as jnp

    cpu = jax.devices("cpu")[0]
    with jax.default_device(cpu):
        loops = jnp.arange(N, dtype=jnp.int32)
        src = jnp.concatenate([jnp.asarray(inputs["edge_index"][0], jnp.int32), loops])
        dst = jnp.concatenate([jnp.asarray(inputs["edge_index"][1], jnp.int32), loops])
        h = jax.nn.relu(jnp.asarray(inputs["x"]) @ jnp.asarray(inputs["Win"])
                        + jnp.asarray(inputs["bin_"]))
        res = h
        for i in range(L):
            xl = h @ jnp.asarray(inputs["Wl"][i]) + jnp.asarray(inputs["bl"][i])
            xr = h @ jnp.asarray(inputs["Wr"][i]) + jnp.asarray(inputs["br"][i])
            e = jax.nn.leaky_relu(xl[src] + xr[dst], NEG)
            s = e @ jnp.asarray(inputs["att"][i])
            m = jax.ops.segment_max(s, dst, num_segments=N)
            w = jnp.exp(s - m[dst])
            z = jax.ops.segment_sum(w, dst, num_segments=N)
            alpha = w / z[dst]
            out = jax.ops.segment_sum(xl[src] * alpha[:, None], dst, num_segments=N) \
                + jnp.asarray(inputs["bg"][i])
            if i > 0:
                out = out + res
            mu = out.mean(-1, keepdims=True)
            var = ((out - mu) ** 2).mean(-1, keepdims=True)
            out = (out - mu) * jax.lax.rsqrt(var + EPS) * jnp.asarray(inputs["ln_g"][i]) \
                + jnp.asarray(inputs["ln_b"][i])
            if i < L - 1:
                out = jax.nn.relu(out)
            h = out
            res = h
        y = jax.nn.relu(h @ jnp.asarray(inputs["W1"]) + jnp.asarray(inputs["b1"])) \
            @ jnp.asarray(inputs["W2"]) + jnp.asarray(inputs["b2"])
        return np.asarray(y, np.float32)


def kernel(x, edge_index, Win, bin_, Wl, bl, Wr, br, att, bg, ln_g, ln_b, W1, b1, W2, b2):
    inputs = dict(x=x, edge_index=edge_index, Win=Win, bin_=bin_, Wl=Wl, bl=bl,
                  Wr=Wr, br=br, att=att, bg=bg, ln_g=ln_g, ln_b=ln_b,
                  W1=W1, b1=b1, W2=W2, b2=b2)
    try:
        # fold the (zero) linear biases in only if nonzero; device path
        # implements bl/br/bg/ln_b=0, ln_g=1 fast path
        for nm in ("bl", "br", "bg", "ln_b"):
            assert not np.any(np.asarray(inputs[nm])), f"{nm} nonzero"
        assert np.all(np.asarray(inputs["ln_g"]) == 1.0), "ln_g != 1"
        meta = _prep_graph(np.asarray(edge_index))
        return _run_device(inputs, meta)
    except Exception as e:  # pragma: no cover
        import traceback
        traceback.print_exc()
        print(f"[kernel] device path failed ({e!r}); host fallback", file=sys.stderr)
        return _host_fallback(inputs)
